# revision 4
# baseline (speedup 1.0000x reference)
"""Normalized-adjacency kernel (EstimateAdj.normalize, symmetric=False) for TRN2.

out = mx * r_inv[:, None] * r_inv[None, :]   where mx = adj + I,
r_inv = rowsum(mx) ** -0.5.

Strategy (8 NeuronCores, row-sharded, raw Bass with explicit semaphores):
  - host: add 1.0 to the diagonal (O(n)), split rows into 8 shards
  - device, per core (shard [1024, 8192], half-tiles [128 x 4096], 16 items):
      pass 1: item 15 is loaded FIRST into a dedicated f32 tile (it has no
              reuse hazard, deepening the initial DMA pipeline); items
              0..14 stream through 3 f32 slots.  Loads alternate between
              the Pool and SP DMA rings.  Each item is consumed by ONE
              scalar-engine activation: Copy with accum_out -> rowsum
              partial, whose `out` writes a bf16 replica into a persistent
              SBUF cache (item 15's Copy is in place, staying f32).  The
              32 MiB shard is cached on-chip and NEVER reloaded from HBM.
      r_inv = 1/sqrt(rowsum), transposed via PE, bf16 via DVE reciprocal.
      The AllGather is SPLIT: AG-a carries rows of tiles 0-6 (ready one
      tile-load before the end of pass 1), AG-b the last tile's 128 rows.
      Cross-core start skew (~25 us, runtime-induced) makes every core
      wait for the slowest at the gather; splitting lets 7/8 of the
      column scales (the "a-columns": first 896 of each 1024-column
      block) land early, so the store phase starts ~15 us sooner.  AG-b
      completes under the a-store stream and gates only the narrow
      b-column strips, drained late through 3 small bounce buffers.
      pass 2: per item, DVE scalar_tensor_tensor over block-strided views:
              slot[:, :, :896]  = (cache * r_inv_row) * colscale_a   (wide)
              bbuf[:, :, :]     = (cache * r_inv_row) * colscale_b   (narrow)
              then block-strided stores; stores alternate SP/Pool rings.
  - bf16 rounding of mx and colscale adds ~8e-3 relative error -- inside
    the 2e-2 gate; rowsums/r_inv row scalars stay f32.
  - host: concatenate the 8 output shards

HBM traffic per core: 32 MiB load + 32 MiB store + ~2 MiB colscale (vs
the two-pass baseline's 92 MiB) -> DMA-bound floor ~190 us at 360 GB/s.
"""

from contextlib import ExitStack

import numpy as np

import concourse.bass as bass
import concourse.mybir as mybir
from concourse.bass_utils import run_bass_kernel_spmd

N = 8192
NCORES = 8
SHARD = N // NCORES  # 1024
P = 128
T = SHARD // P  # 8 tiles per core
H = 2  # column halves per tile
W = N // H  # 4096
B = 4  # 1024-col blocks per half
V = W // B  # 1024
VA = SHARD - P  # 896 "a" columns per block (tiles 0-6 rows)
VB = P  # 128 "b" columns per block (tile 7 rows)

F32 = mybir.dt.float32
BF16 = mybir.dt.bfloat16
NSLOTS = 3  # f32 streaming slots (pass-1 loads / pass-2 wide outputs)
NB = 3  # f32 bounce buffers for the narrow b-column outputs


def build_kernel():
    items = [(t, h) for t in range(T) for h in range(H)]
    ni = len(items)
    last = ni - 1  # item 15: dedicated f32 tile, scaled in place
    lorder = [last] + list(range(ni - 1))  # load + scalar-engine order
    groups = [(0, T - 1), (T - 1, T)]
    ng = len(groups)

    nc = bass.Bass(num_devices=NCORES)
    mx = nc.dram_tensor("mx", [SHARD, N], F32, kind="ExternalInput")
    eye = nc.dram_tensor("eye", [P, P], F32, kind="ExternalInput")
    out = nc.dram_tensor("out", [SHARD, N], F32, kind="ExternalOutput")
    cc_in_a = nc.dram_tensor("cc_in_a", [VA], BF16)
    cc_in_b = nc.dram_tensor("cc_in_b", [VB], BF16)
    cc_out_a = nc.dram_tensor("cc_out_a", [NCORES, VA], BF16, addr_space="Shared")
    cc_out_b = nc.dram_tensor("cc_out_b", [NCORES, VB], BF16, addr_space="Shared")

    # tile t, partition p, half h, block b, col v -> shard row t*128 + p,
    # col h*4096 + b*1024 + v
    mx_v = mx.rearrange("(t p) (h w) -> t p h w", p=P, h=H)
    out_v = out.rearrange("(t p) (h b v) -> t p h b v", p=P, h=H, b=B)

    with ExitStack() as ctx:
        slots = [
            ctx.enter_context(nc.sbuf_tensor(f"slot{s}", [P, B, V], F32))
            for s in range(NSLOTS)
        ]
        ded = ctx.enter_context(nc.sbuf_tensor("ded", [P, B, V], F32))
        cache = [
            ctx.enter_context(nc.sbuf_tensor(f"cache{i}", [P, B, V], BF16))
            for i in range(ni - 1)
        ]
        colscale = ctx.enter_context(
            nc.sbuf_tensor("colscale", [P, NCORES, V], BF16)
        )
        bbuf = [
            ctx.enter_context(nc.sbuf_tensor(f"bbuf{j}", [P, B, VB], F32))
            for j in range(NB)
        ]
        eye_sb = ctx.enter_context(nc.sbuf_tensor("eye_sb", [P, P], F32))
        ps = ctx.enter_context(nc.sbuf_tensor("ps", [P, ni], F32))
        rs = ctx.enter_context(nc.sbuf_tensor("rs", [P, T], F32))
        rinv = ctx.enter_context(nc.sbuf_tensor("rinv", [P, T], F32))
        ptc = [
            ctx.enter_context(nc.sbuf_tensor(f"ptc{g}", [b - a, P], BF16))
            for g, (a, b) in enumerate(groups)
        ]
        pt = [
            ctx.enter_context(nc.psum_tensor(f"pt{g}", [b - a, P], F32))
            for g, (a, b) in enumerate(groups)
        ]

        s_in = [
            ctx.enter_context(nc.semaphore(f"s_in{s}")) for s in range(NSLOTS)
        ]
        s_ind = ctx.enter_context(nc.semaphore("s_ind"))  # ded-tile load
        s_sout = [
            ctx.enter_context(nc.semaphore(f"s_sout{s}")) for s in range(NSLOTS)
        ]
        s_soutd = ctx.enter_context(nc.semaphore("s_soutd"))  # ded store
        s_soutb = [
            ctx.enter_context(nc.semaphore(f"s_soutb{j}")) for j in range(NB)
        ]
        s_red = ctx.enter_context(nc.semaphore("s_red"))
        s_eye = ctx.enter_context(nc.semaphore("s_eye"))
        s_cmb = [
            ctx.enter_context(nc.semaphore(f"s_cmb{g}")) for g in range(ng)
        ]
        s_sqrt = [
            ctx.enter_context(nc.semaphore(f"s_sqrt{g}")) for g in range(ng)
        ]
        s_rcp = ctx.enter_context(nc.semaphore("s_rcp"))
        s_tp = [ctx.enter_context(nc.semaphore(f"s_tp{g}")) for g in range(ng)]
        s_ptc = [
            ctx.enter_context(nc.semaphore(f"s_ptc{g}")) for g in range(ng)
        ]
        s_ccin_a = ctx.enter_context(nc.semaphore("s_ccin_a"))
        s_ccin_b = ctx.enter_context(nc.semaphore("s_ccin_b"))
        s_cc_a = ctx.enter_context(nc.semaphore("s_cc_a"))
        s_cc_b = ctx.enter_context(nc.semaphore("s_cc_b"))
        s_cs_a = ctx.enter_context(nc.semaphore("s_cs_a"))
        s_cs_b = ctx.enter_context(nc.semaphore("s_cs_b"))
        s_stt = ctx.enter_context(nc.semaphore("s_stt"))
        block = ctx.enter_context(nc.Block())

        def load_src(i):
            t, h = items[i]
            return mx_v[t, :, h]

        def in_tile(i):
            return ded if i == last else slots[i % NSLOTS]

        def in_sem_val(i):
            if i == last:
                return s_ind, 16
            return s_in[i % NSLOTS], 16 * (i // NSLOTS + 1)

        def cache_of(i):
            return ded if i == last else cache[i]

        # Load order: [15, 0, 1, ..., 14]; even positions Pool, odd SP.
        # Slot reuse is safe across rings: the load of item i waits until
        # the scalar engine consumed item i-3 (s_red in lorder counting),
        # which transitively orders each slot's s_in increments.
        def emit_loads(eng, parity):
            for j, i in enumerate(lorder):
                if j % 2 != parity:
                    continue
                if i != last and i >= NSLOTS:
                    # ACT pos of item i-3 is i-2 (item 15 occupies pos 0)
                    eng.wait_ge(s_red, i - 1)
                sem, _ = in_sem_val(i)
                eng.dma_start(in_tile(i)[:, :, :], load_src(i)).then_inc(
                    sem, 16
                )

        @block.gpsimd
        def _(g):
            emit_loads(g, 0)
            # local r_inv of tiles 0-6 (transposed, bf16) -> DRAM
            g.wait_ge(s_ptc[0], 1)
            g.dma_start(cc_in_a[:], ptc[0][:, :]).then_inc(s_ccin_a, 16)
            g.wait_ge(s_ccin_a, 16)
            g.collective_compute(
                "AllGather",
                mybir.AluOpType.bypass,
                replica_groups=[list(range(NCORES))],
                ins=[cc_in_a[:]],
                outs=[cc_out_a[:, :]],
            ).then_inc(s_cc_a, 1)
            g.wait_ge(s_ccin_b, 16)
            g.collective_compute(
                "AllGather",
                mybir.AluOpType.bypass,
                replica_groups=[list(range(NCORES))],
                ins=[cc_in_b[:]],
                outs=[cc_out_b[:, :]],
            ).then_inc(s_cc_b, 1)
            # column-scale broadcasts (block-strided dsts)
            g.wait_ge(s_cc_a, 1)
            g.dma_start(
                colscale[:, :, 0:VA],
                cc_out_a[:, :].partition_broadcast(P),
            ).then_inc(s_cs_a, 16)
            g.wait_ge(s_cc_b, 1)
            g.dma_start(
                colscale[:, :, VA:V],
                cc_out_b[:, :].partition_broadcast(P),
            ).then_inc(s_cs_b, 16)
            # odd-k wide stores, then odd-k narrow stores
            for k in range(1, ni, 2):
                t, h = items[k]
                g.wait_ge(s_stt, k + 1)
                sem = s_soutd if k == last else s_sout[k % NSLOTS]
                g.dma_start(
                    out_v[t, :, h, :, 0:VA],
                    (ded if k == last else slots[k % NSLOTS])[:, :, 0:VA],
                ).then_inc(sem, 16)
            for k in range(1, ni, 2):
                t, h = items[k]
                g.wait_ge(s_stt, ni + k + 1)
                g.dma_start(
                    out_v[t, :, h, :, VA:V], bbuf[k % NB][:, :, :]
                ).then_inc(s_soutb[k % NB], 16)

        @block.sync
        def _(sp):
            # first SP loads go out before the eye (PE needs it only at
            # ~115 us); emit_loads with a small prefix split
            for j, i in enumerate(lorder):
                if j % 2 != 1:
                    continue
                if j == 5:
                    sp.dma_start(eye_sb[:, :], eye[:, :]).then_inc(s_eye, 16)
                if i != last and i >= NSLOTS:
                    sp.wait_ge(s_red, i - 1)
                sem, _ = in_sem_val(i)
                sp.dma_start(in_tile(i)[:, :, :], load_src(i)).then_inc(
                    sem, 16
                )
            # local r_inv of tile 7 -> DRAM
            sp.wait_ge(s_ptc[1], 1)
            sp.dma_start(cc_in_b[:], ptc[1][:, :]).then_inc(s_ccin_b, 16)
            # even-k wide stores, then even-k narrow stores
            for k in range(0, ni, 2):
                t, h = items[k]
                sp.wait_ge(s_stt, k + 1)
                sp.dma_start(
                    out_v[t, :, h, :, 0:VA], slots[k % NSLOTS][:, :, 0:VA]
                ).then_inc(s_sout[k % NSLOTS], 16)
            for k in range(0, ni, 2):
                t, h = items[k]
                sp.wait_ge(s_stt, ni + k + 1)
                sp.dma_start(
                    out_v[t, :, h, :, VA:V], bbuf[k % NB][:, :, :]
                ).then_inc(s_soutb[k % NB], 16)
            # all stores landed before halt
            for s in range(NSLOTS):
                sp.wait_ge(s_sout[s], 16 * 5)
            sp.wait_ge(s_soutd, 16)
            bcnt = [0] * NB
            for k in range(ni):
                bcnt[k % NB] += 16
            for j in range(NB):
                sp.wait_ge(s_soutb[j], bcnt[j])

        @block.scalar
        def _(s):
            # pass 1 in lorder: rowsum partials via Copy-with-accum; the
            # Copy output IS the bf16 cache write (item 15 copies in place)
            emitted = 0
            for gi, (a, b) in enumerate(groups):
                # finish all items of tiles < b (item 15 is tile 7; it is
                # position 0 and only group 1 requires it)
                need = [last] + list(range(b * H - (1 if gi == ng - 1 else 0)))
                for i in need[emitted:]:
                    sem, val = in_sem_val(i)
                    s.wait_ge(sem, val)
                    s.activation(
                        cache_of(i)[:, :, :],
                        in_tile(i)[:, :, :],
                        mybir.ActivationFunctionType.Copy,
                        accum_out=ps[:, i : i + 1],
                    ).then_inc(s_red, 1)
                emitted = len(need)
                if b - a == 1:
                    # single-tile group: fused halves-combine + sqrt
                    # (self-wait drains this engine's accum writebacks)
                    s.wait_ge(s_red, ni)
                    s.activation(
                        rs[:, a:b],
                        ps[:, 2 * a : 2 * a + 1],
                        mybir.ActivationFunctionType.Sqrt,
                        bias=ps[:, 2 * a + 1 : 2 * a + 2],
                        scale=1.0,
                    ).then_inc(s_sqrt[gi], 1)
                else:
                    s.wait_ge(s_cmb[gi], 1)
                    s.sqrt(rs[:, a:b], rs[:, a:b]).then_inc(s_sqrt[gi], 1)

        @block.tensor
        def _(pe):
            # sqrt(rowsum) [128, g] -> [g, 128] in PSUM (via identity)
            pe.wait_ge(s_eye, 16)
            for gi, (a, b) in enumerate(groups):
                pe.wait_ge(s_sqrt[gi], 1)
                pe.transpose(pt[gi][:, :], rs[:, a:b], eye_sb[:, :]).then_inc(
                    s_tp[gi], 1
                )

        @block.vector
        def _(v):
            assert H == 2
            for gi, (a, b) in enumerate(groups):
                if b - a > 1:
                    # combine halves: rs[:, t] = sum_h ps[:, t*H + h];
                    # items 0..13 are ACT positions 1..14 -> s_red >= 15
                    v.wait_ge(s_red, b * H + 1)
                    v.scalar_tensor_tensor(
                        rs[:, a:b],
                        ps[:, 2 * a : 2 * b : 2],
                        1.0,
                        ps[:, 2 * a + 1 : 2 * b : 2],
                        op0=mybir.AluOpType.mult,
                        op1=mybir.AluOpType.add,
                    ).then_inc(s_cmb[gi], 1)
                # row-scalar r_inv (f32) for the pass-2 scales
                v.wait_ge(s_sqrt[gi], 1)
                v.reciprocal(rinv[:, a:b], rs[:, a:b]).then_inc(s_rcp, 1)
                # transposed r_inv in bf16, ready for the cc DRAM write
                v.wait_ge(s_tp[gi], 1)
                with nc.allow_low_precision("bf16 column scale is in-gate"):
                    v.reciprocal(ptc[gi][:, :], pt[gi][:, :]).then_inc(
                        s_ptc[gi], 1
                    )
            # pass 2, wide a-columns: slot = (cache * rinv_row) * colscale_a
            v.wait_ge(s_rcp, ng)
            v.wait_ge(s_cs_a, 16)
            for k in range(ni):
                t, h = items[k]
                if NSLOTS <= k < last:
                    v.wait_ge(s_sout[k % NSLOTS], 16 * (k // NSLOTS))
                v.scalar_tensor_tensor(
                    (ded if k == last else slots[k % NSLOTS])[:, :, 0:VA],
                    cache_of(k)[:, :, 0:VA],
                    rinv[:, t : t + 1],
                    colscale[:, B * h : B * (h + 1), 0:VA],
                    op0=mybir.AluOpType.mult,
                    op1=mybir.AluOpType.mult,
                ).then_inc(s_stt, 1)
            # narrow b-columns through the bounce buffers
            v.wait_ge(s_cs_b, 16)
            for k in range(ni):
                t, h = items[k]
                if k >= NB:
                    v.wait_ge(s_soutb[k % NB], 16 * (k // NB))
                v.scalar_tensor_tensor(
                    bbuf[k % NB][:, :, :],
                    cache_of(k)[:, :, VA:V],
                    rinv[:, t : t + 1],
                    colscale[:, B * h : B * (h + 1), VA:V],
                    op0=mybir.AluOpType.mult,
                    op1=mybir.AluOpType.mult,
                ).then_inc(s_stt, 1)

    return nc


_NC_CACHE = {}


def _get_nc():
    if "nc" not in _NC_CACHE:
        _NC_CACHE["nc"] = build_kernel()
    return _NC_CACHE["nc"]


def kernel(adj, **run_kwargs):
    adj = np.asarray(adj)
    assert adj.shape == (N, N) and adj.dtype == np.float32
    mx = adj.copy()
    idx = np.arange(N)
    mx[idx, idx] += 1.0
    eye = np.eye(P, dtype=np.float32)

    in_maps = [
        {"mx": mx[c * SHARD : (c + 1) * SHARD], "eye": eye}
        for c in range(NCORES)
    ]
    nc = _get_nc()
    try:
        res = run_bass_kernel_spmd(nc, in_maps, list(range(NCORES)), **run_kwargs)
    except Exception:
        # transient device hiccups (e.g. a wedged core from an earlier
        # process) sometimes clear on a second attempt
        import time

        time.sleep(2.0)
        res = run_bass_kernel_spmd(nc, in_maps, list(range(NCORES)), **run_kwargs)
    out = np.concatenate([res.results[c]["out"] for c in range(NCORES)], axis=0)
    if run_kwargs:
        return out, res
    return out


# revision 13
# speedup vs baseline: 1.1691x; 1.1691x over previous
"""Normalized-adjacency kernel (EstimateAdj.normalize, symmetric=False) for TRN2.

out = mx * r_inv[:, None] * r_inv[None, :]   where mx = adj + I,
r_inv = rowsum(mx) ** -0.5.

Strategy (8 NeuronCores, row-sharded, raw Bass with explicit semaphores):
  - host: add 1.0 to the diagonal (O(n)), split rows into 8 shards
  - device, per core (shard [1024, 8192], half-tiles [128 x 4096], 16 items):
      pass 1: item 15 is loaded FIRST into a dedicated f32 tile (no reuse
              hazard -> deeper initial DMA pipeline); items 0..14 stream
              through 3 f32 slots, loads alternating Pool/SP DMA rings.
              Each item is consumed by ONE scalar-engine activation (Copy
              with accum_out): the accumulator is the rowsum partial and
              the Copy output writes a bf16 replica into a persistent SBUF
              cache (item 15 copies in place, staying f32).  The 32 MiB
              shard is cached on-chip and NEVER reloaded from HBM.
      r_inv = 1/sqrt(rowsum), transposed via PE, bf16 via DVE reciprocal.
      The AllGather is SPLIT BY ROW HALVES to bury both the ~12 us
      first-collective setup and the ~25 us cross-core start skew (the
      runtime staggers core launches; every core would otherwise idle at
      the gather waiting for the slowest):
        AG-a: r_inv of tiles 0-3 -- its input is ready at ~56% of the
              load phase, so setup + skew complete BEFORE loads finish.
        AG-b: tiles 4-7, triggered right after the last rowsum; the CC
              pipeline is warm by then (second-collective setup ~2 us)
              and it completes far before the b-columns are needed.
      Triggers and cc_in writes are issued from the Scalar engine's DMA
      queue (empty, no queueing behind bulk loads); the colscale
      broadcasts from DVE's own queue.
      pass 2: per item, two DVE scalar_tensor_tensor ops over
      block-strided views (columns j with (j mod 1024) < 512 are
      "a-columns" = tiles 0-3 rows; the rest "b-columns"):
              slot[:, :, :512] = (cache * r_inv_row) * colscale_a
              slot[:, :, 512:] = (cache * r_inv_row) * colscale_b
      and two block-strided stores (2 KB descriptors), alternating
      SP/Pool rings.  a-stores start as soon as the core's own loads
      finish (~125 us); all b-work hides under the a-store stream.
  - bf16 rounding of mx and colscale adds ~8e-3 relative error -- inside
    the 2e-2 gate; rowsums/r_inv row scalars stay f32.
  - host: concatenate the 8 output shards

HBM traffic per core: 32 MiB load + 32 MiB store + 2 MiB colscale (vs the
two-pass baseline's 92 MiB) -> DMA-bound floor ~190 us at 360 GB/s.
"""

from contextlib import ExitStack

import numpy as np

import concourse.bass as bass
import concourse.mybir as mybir
from concourse.bass_utils import run_bass_kernel_spmd

N = 8192
NCORES = 8
SHARD = N // NCORES  # 1024
P = 128
T = SHARD // P  # 8 tiles per core
H = 2  # column halves per tile
W = N // H  # 4096
B = 4  # 1024-col blocks per half
V = W // B  # 1024
VA = V // 2  # 512 "a" columns per block (tiles 0-3 rows)

F32 = mybir.dt.float32
BF16 = mybir.dt.bfloat16
NSLOTS = 3  # f32 streaming slots (pass-1 loads / pass-2 outputs)


def build_kernel():
    items = [(t, h) for t in range(T) for h in range(H)]
    ni = len(items)
    last = ni - 1  # item 15: dedicated f32 tile, scaled in place
    lorder = [last] + list(range(ni - 1))  # load + scalar-engine order
    groups = [(0, T // 2), (T // 2, T)]
    ng = len(groups)

    nc = bass.Bass(num_devices=NCORES)
    mx = nc.dram_tensor("mx", [SHARD, N], F32, kind="ExternalInput")
    eye = nc.dram_tensor("eye", [P, P], F32, kind="ExternalInput")
    out = nc.dram_tensor("out", [SHARD, N], F32, kind="ExternalOutput")
    cc_in = [
        nc.dram_tensor(f"cc_in_{x}", [SHARD // 2], BF16) for x in ("a", "b")
    ]
    cc_out = [
        nc.dram_tensor(
            f"cc_out_{x}", [NCORES, SHARD // 2], BF16, addr_space="Shared"
        )
        for x in ("a", "b")
    ]

    # tile t, partition p, half h, block b, col v -> shard row t*128 + p,
    # col h*4096 + b*1024 + v
    mx_v = mx.rearrange("(t p) (h w) -> t p h w", p=P, h=H)
    out_v = out.rearrange("(t p) (h b v) -> t p h b v", p=P, h=H, b=B)

    with ExitStack() as ctx:
        slots = [
            ctx.enter_context(nc.sbuf_tensor(f"slot{s}", [P, B, V], F32))
            for s in range(NSLOTS)
        ]
        ded = ctx.enter_context(nc.sbuf_tensor("ded", [P, B, V], F32))
        cache = [
            ctx.enter_context(nc.sbuf_tensor(f"cache{i}", [P, B, V], BF16))
            for i in range(ni - 1)
        ]
        colscale = ctx.enter_context(
            nc.sbuf_tensor("colscale", [P, NCORES, V], BF16)
        )
        eye_sb = ctx.enter_context(nc.sbuf_tensor("eye_sb", [P, P], F32))
        ps = ctx.enter_context(nc.sbuf_tensor("ps", [P, ni], F32))
        rs = ctx.enter_context(nc.sbuf_tensor("rs", [P, T], F32))
        rinv = ctx.enter_context(nc.sbuf_tensor("rinv", [P, T], F32))
        ptc = [
            ctx.enter_context(nc.sbuf_tensor(f"ptc{g}", [b - a, P], BF16))
            for g, (a, b) in enumerate(groups)
        ]
        pt = [
            ctx.enter_context(nc.psum_tensor(f"pt{g}", [b - a, P], F32))
            for g, (a, b) in enumerate(groups)
        ]

        s_in = [
            ctx.enter_context(nc.semaphore(f"s_in{s}")) for s in range(NSLOTS)
        ]
        s_ind = ctx.enter_context(nc.semaphore("s_ind"))  # ded-tile load
        s_sout = [
            ctx.enter_context(nc.semaphore(f"s_sout{s}")) for s in range(NSLOTS)
        ]
        s_soutb = [
            ctx.enter_context(nc.semaphore(f"s_soutb{s}"))
            for s in range(NSLOTS)
        ]
        s_soutd = ctx.enter_context(nc.semaphore("s_soutd"))  # ded stores
        s_red = ctx.enter_context(nc.semaphore("s_red"))
        s_eye = ctx.enter_context(nc.semaphore("s_eye"))
        s_cmb = [
            ctx.enter_context(nc.semaphore(f"s_cmb{g}")) for g in range(ng)
        ]
        s_sqrt = [
            ctx.enter_context(nc.semaphore(f"s_sqrt{g}")) for g in range(ng)
        ]
        s_tp = [ctx.enter_context(nc.semaphore(f"s_tp{g}")) for g in range(ng)]
        s_ptc = [
            ctx.enter_context(nc.semaphore(f"s_ptc{g}")) for g in range(ng)
        ]
        s_ccin = [
            ctx.enter_context(nc.semaphore(f"s_ccin{g}")) for g in range(ng)
        ]
        s_cc = [ctx.enter_context(nc.semaphore(f"s_cc{g}")) for g in range(ng)]
        s_cs = [ctx.enter_context(nc.semaphore(f"s_cs{g}")) for g in range(ng)]
        s_stt = ctx.enter_context(nc.semaphore("s_stt"))
        block = ctx.enter_context(nc.Block())

        def load_src(i):
            t, h = items[i]
            return mx_v[t, :, h]

        def in_tile(i):
            return ded if i == last else slots[i % NSLOTS]

        def in_sem_val(i):
            if i == last:
                return s_ind, 16
            return s_in[i % NSLOTS], 16 * (i // NSLOTS + 1)

        def cache_of(i):
            return ded if i == last else cache[i]

        def span(x):  # column range of phase x within each block
            return (0, VA) if x == 0 else (VA, V)

        # Load order: [15, 0, 1, ..., 14]; even positions Pool, odd SP.
        # Slot reuse is safe across rings: the load of item i waits until
        # the scalar engine consumed item i-3 (ACT position i-1), which
        # transitively orders each slot's s_in increments.
        def emit_loads(eng, parity):
            for j, i in enumerate(lorder):
                if j % 2 != parity:
                    continue
                if parity == 1 and j == 5:
                    # eye rides the SP ring early; PE needs it at ~110 us
                    eng.dma_start(eye_sb[:, :], eye[:, :]).then_inc(s_eye, 16)
                if i != last and i >= NSLOTS:
                    eng.wait_ge(s_red, i - 1)
                sem, _ = in_sem_val(i)
                eng.dma_start(in_tile(i)[:, :, :], load_src(i)).then_inc(
                    sem, 16
                )

        def emit_stores(eng, parity, x, sttbase, souts):
            lo, hi = span(x)
            for k in range(parity, ni, 2):
                t, h = items[k]
                eng.wait_ge(s_stt, sttbase + k + 1)
                sem = s_soutd if k == last else souts[k % NSLOTS]
                src = (ded if k == last else slots[k % NSLOTS])[:, :, lo:hi]
                eng.dma_start(out_v[t, :, h, :, lo:hi], src).then_inc(sem, 16)

        def gather(g, gi):
            # NRT requires straight-line collective ordering on gpsimd
            g.wait_ge(s_ccin[gi], 16)
            g.collective_compute(
                "AllGather",
                mybir.AluOpType.bypass,
                replica_groups=[list(range(NCORES))],
                ins=[cc_in[gi][:]],
                outs=[cc_out[gi][:, :]],
            ).then_inc(s_cc[gi], 1)

        @block.gpsimd
        def _(g):
            for j, i in enumerate(lorder):
                if j % 2 != 0:
                    continue
                if i != last and i >= NSLOTS:
                    g.wait_ge(s_red, i - 1)
                sem, _ = in_sem_val(i)
                g.dma_start(in_tile(i)[:, :, :], load_src(i)).then_inc(
                    sem, 16
                )
                if j == 10:
                    # AG-a trigger: its ~12 us setup and the cross-core
                    # start skew hide under the remaining load tail; the
                    # SP ring keeps the DMA engines fed during the wait
                    gather(g, 0)
            # first a-store, then the (warm, ~2 us) AG-b trigger
            for k in (1,):
                t, h = items[k]
                g.wait_ge(s_stt, k + 1)
                g.dma_start(
                    out_v[t, :, h, :, 0:VA],
                    slots[k % NSLOTS][:, :, 0:VA],
                ).then_inc(s_sout[k % NSLOTS], 16)
            gather(g, 1)
            lo, hi = span(0)
            for k in range(3, ni, 2):
                t, h = items[k]
                g.wait_ge(s_stt, k + 1)
                sem = s_soutd if k == last else s_sout[k % NSLOTS]
                src = (ded if k == last else slots[k % NSLOTS])[:, :, lo:hi]
                g.dma_start(out_v[t, :, h, :, lo:hi], src).then_inc(sem, 16)
            emit_stores(g, 1, 1, ni, s_soutb)

        @block.sync
        def _(sp):
            emit_loads(sp, 1)
            emit_stores(sp, 0, 0, 0, s_sout)
            emit_stores(sp, 0, 1, ni, s_soutb)
            # all stores landed before halt
            for s in range(NSLOTS):
                sp.wait_ge(s_sout[s], 16 * 5)
                sp.wait_ge(s_soutb[s], 16 * 5)
            sp.wait_ge(s_soutd, 32)

        @block.scalar
        def _(s):
            # pass 1 in lorder: rowsum partials via Copy-with-accum; the
            # Copy output IS the bf16 cache write (item 15 copies in place).
            # The AG triggers ride this engine's (otherwise empty) queue.
            def red(i):
                sem, val = in_sem_val(i)
                s.wait_ge(sem, val)
                s.activation(
                    cache_of(i)[:, :, :],
                    in_tile(i)[:, :, :],
                    mybir.ActivationFunctionType.Copy,
                    accum_out=ps[:, i : i + 1],
                ).then_inc(s_red, 1)

            red(last)
            for i in range(0, T * H // 2):  # items of tiles 0-3
                red(i)
            s.wait_ge(s_cmb[0], 1)
            s.sqrt(rs[:, 0 : T // 2], rs[:, 0 : T // 2]).then_inc(s_sqrt[0], 1)
            s.wait_ge(s_ptc[0], 1)
            s.dma_start(cc_in[0][:], ptc[0][:, :]).then_inc(s_ccin[0], 16)
            def bcast(gi):
                lo, hi = span(gi)
                s.wait_ge(s_cc[gi], 1)
                s.dma_start(
                    colscale[:, :, lo:hi],
                    cc_out[gi][:, :].partition_broadcast(P),
                ).then_inc(s_cs[gi], 16)

            for i in range(8, ni - 3):
                red(i)
            # AG-a completes ~4 us before the earliest core reaches this
            # point; issuing the broadcast here (empty ACT DMA ring) gets
            # colscale_a ready right as the first slot frees up
            bcast(0)
            red(ni - 3)
            red(ni - 2)
            s.wait_ge(s_cmb[1], 1)
            s.sqrt(rs[:, T // 2 : T], rs[:, T // 2 : T]).then_inc(s_sqrt[1], 1)
            s.wait_ge(s_ptc[1], 1)
            s.dma_start(cc_in[1][:], ptc[1][:, :]).then_inc(s_ccin[1], 16)
            bcast(1)

        @block.tensor
        def _(pe):
            # sqrt(rowsum) [128, g] -> [g, 128] in PSUM (via identity)
            pe.wait_ge(s_eye, 16)
            for gi, (a, b) in enumerate(groups):
                pe.wait_ge(s_sqrt[gi], 1)
                pe.transpose(pt[gi][:, :], rs[:, a:b], eye_sb[:, :]).then_inc(
                    s_tp[gi], 1
                )

        @block.vector
        def _(v):
            assert H == 2

            def chain(gi):
                a, b = groups[gi]
                # items a*H..b*H-1 sit at ACT positions a*H+2..b*H+1
                # (item 15 holds position 1)
                v.wait_ge(s_red, min(b * H + 1, ni))
                v.scalar_tensor_tensor(
                    rs[:, a:b],
                    ps[:, 2 * a : 2 * b : 2],
                    1.0,
                    ps[:, 2 * a + 1 : 2 * b : 2],
                    op0=mybir.AluOpType.mult,
                    op1=mybir.AluOpType.add,
                ).then_inc(s_cmb[gi], 1)
                v.wait_ge(s_sqrt[gi], 1)
                v.reciprocal(rinv[:, a:b], rs[:, a:b])
                v.wait_ge(s_tp[gi], 1)
                with nc.allow_low_precision("bf16 column scale is in-gate"):
                    v.reciprocal(ptc[gi][:, :], pt[gi][:, :]).then_inc(
                        s_ptc[gi], 1
                    )

            def stt(k, x, souts):
                t, h = items[k]
                lo, hi = span(x)
                if x == 0 and k < NSLOTS:
                    # slot k's last pass-1 occupant is item 12+k (ACT
                    # position 14+k); don't overwrite before it is read
                    v.wait_ge(s_red, 14 + k)
                if NSLOTS <= k < last:
                    v.wait_ge(souts[k % NSLOTS], 16 * (k // NSLOTS))
                v.scalar_tensor_tensor(
                    (ded if k == last else slots[k % NSLOTS])[:, :, lo:hi],
                    cache_of(k)[:, :, lo:hi],
                    rinv[:, t : t + 1],
                    colscale[:, B * h : B * (h + 1), lo:hi],
                    op0=mybir.AluOpType.mult,
                    op1=mybir.AluOpType.mult,
                ).then_inc(s_stt, 1)

            chain(0)
            v.wait_ge(s_cs[0], 16)
            stt(0, 0, s_sout)
            stt(1, 0, s_sout)
            # the g1 r_inv chain rides between early a-stts: its cc_in_b
            # write (ACT) then triggers AG-b while a-stores stream
            chain(1)
            for k in range(2, ni):
                stt(k, 0, s_sout)
            v.wait_ge(s_cs[1], 16)
            for k in range(ni):
                stt(k, 1, s_soutb)

    return nc


_NC_CACHE = {}


def _get_nc():
    if "nc" not in _NC_CACHE:
        _NC_CACHE["nc"] = build_kernel()
    return _NC_CACHE["nc"]


def kernel(adj, **run_kwargs):
    adj = np.asarray(adj)
    assert adj.shape == (N, N) and adj.dtype == np.float32
    mx = adj.copy()
    idx = np.arange(N)
    mx[idx, idx] += 1.0
    eye = np.eye(P, dtype=np.float32)

    in_maps = [
        {"mx": mx[c * SHARD : (c + 1) * SHARD], "eye": eye}
        for c in range(NCORES)
    ]
    nc = _get_nc()
    try:
        res = run_bass_kernel_spmd(nc, in_maps, list(range(NCORES)), **run_kwargs)
    except Exception:
        # transient device hiccups (e.g. a wedged core from an earlier
        # process) sometimes clear on a second attempt
        import time

        time.sleep(2.0)
        res = run_bass_kernel_spmd(nc, in_maps, list(range(NCORES)), **run_kwargs)
    out = np.concatenate([res.results[c]["out"] for c in range(NCORES)], axis=0)
    if run_kwargs:
        return out, res
    return out
